# revision 8
# baseline (speedup 1.0000x reference)
"""Fused multi-head-attention block (QKV proj -> attention -> out proj ->
residual -> LayerNorm) for Trainium2, SPMD over 8 NeuronCores.

Sharding: (batch, query-half). Core c handles batch b = c//2 and query rows
[qh*S/2, (qh+1)*S/2) with qh = c%2. Each core computes K/V over the full
sequence (recomputed per query-half; cheap) and its query half end-to-end,
so no cross-core communication is needed.

Returns (out, attn) exactly like the reference nn.Module.
"""

import numpy as np

import concourse.bacc as bacc
import concourse.mybir as mybir
import concourse.tile as tile
from concourse.bass import ts
from concourse.bass_utils import run_bass_kernel_spmd
from concourse.masks import make_identity

F32 = mybir.dt.float32
F32R = mybir.dt.float32r

B, S, D, H = 4, 2048, 256, 4
HD = D // H  # 64
N_CORES = 8
P = 128


def build_program(seq=S, n_cores=N_CORES, repeats=1):
    """Build the per-core program. seq = full sequence length; each core
    handles sq = seq//2 query rows. `repeats` re-runs the whole computation
    (for timing by differencing)."""
    sq = seq // 2
    assert seq % 1024 == 0, "supported seq multiples of 1024"
    NK = seq // P   # k blocks over full seq
    NQ = sq // P    # q blocks for this core's half
    KC = seq // 512  # 512-wide k chunks
    QC = sq // 512   # 512-wide q chunks

    nc = bacc.Bacc("TRN2", target_bir_lowering=False, debug=False,
                   num_devices=n_cores)

    x_d = nc.declare_dram_parameter("x", [seq, D], F32, isOutput=False)
    xq_d = nc.declare_dram_parameter("xq", [sq, D], F32, isOutput=False)
    w_d = {}
    for w in ("Wq", "Wk", "Wv", "Wo"):
        w_d[w] = nc.declare_dram_parameter(w, [D, D], F32, isOutput=False)
    v_d = {}
    for v in ("bq", "bk", "bv", "bo", "gamma", "beta"):
        v_d[v] = nc.declare_dram_parameter(v, [1, D], F32, isOutput=False)
    attn_d = nc.declare_dram_parameter("attn_p", [H, sq, seq], F32, isOutput=True)
    out_d = nc.declare_dram_parameter("out_p", [sq, D], F32, isOutput=True)

    with tile.TileContext(nc) as tc:
        with (
            tc.tile_pool(name="consts", bufs=1) as consts,
            tc.tile_pool(name="io", bufs=1) as io,
            tc.tile_pool(name="attnp", bufs=3) as attnp,
            tc.tile_pool(name="ets", bufs=4) as etsp,
            tc.tile_pool(name="stat", bufs=6) as stat,
            tc.tile_pool(name="rec", bufs=1) as recp,
            tc.tile_pool(name="yac", bufs=2) as yacp,
            tc.tile_pool(name="ps_s", bufs=2, space="PSUM") as ps_s,
            tc.tile_pool(name="ps_et", bufs=2, space="PSUM") as ps_et,
        ):
            # ---- constants -------------------------------------------------
            ident = consts.tile([P, P], F32)
            make_identity(nc, ident)
            ones_row = consts.tile([1, P], F32)
            nc.vector.memset(ones_row, 1.0)
            eps_t = consts.tile([P, 1], F32)
            nc.vector.memset(eps_t, 1e-5)

            w_sb = {}
            for w in ("Wq", "Wk", "Wv", "Wo"):
                traw = io.tile([P, 2, D], F32, tag=f"raw_{w}")
                nc.sync.dma_start(traw[:], w_d[w].rearrange("(t p) n -> p t n", p=P))
                t = io.tile([P, 2, D], F32R, tag=f"sb_{w}")
                nc.vector.tensor_copy(t[:], traw[:])
                w_sb[w] = t
            v_row = {}
            for v in ("bq", "bk", "bv", "bo", "gamma", "beta"):
                t = io.tile([1, D], F32, tag=f"row_{v}")
                nc.sync.dma_start(t[:], v_d[v][:])
                v_row[v] = t

            x_sb = io.tile([P, NK, D], F32, tag="x_sb")
            nc.sync.dma_start(x_sb[:], x_d.rearrange("(t p) d -> p t d", p=P))
            xq_sb = io.tile([P, NQ, D], F32, tag="xq_sb")
            nc.sync.dma_start(xq_sb[:], xq_d.rearrange("(t p) d -> p t d", p=P))

            # per-partition bias columns for Q/K (bias indexed by output dim,
            # which lands on partitions in the QT/KT layout), via K=1 matmul
            bqT = consts.tile([P, 2], F32)
            bkT = consts.tile([P, 2], F32)
            for pair in range(2):
                for vec, dst in (("bq", bqT), ("bk", bkT)):
                    pp = ps_s.tile([P, 1024], F32, tag="ps")
                    nc.tensor.matmul(pp[:, 0:1],
                                     v_row[vec][0:1, ts(pair, P)],
                                     ones_row[0:1, 0:1], start=True, stop=True)
                    nc.vector.tensor_copy(dst[:, pair:pair + 1], pp[:, 0:1])
            # broadcast rows -> [P, D] tiles (bias/gamma/beta along free dim)
            bcast = {}
            for vec in ("bo", "gamma", "beta", "bv"):
                t = consts.tile([P, D], F32, tag=f"bc_{vec}")
                pp = ps_s.tile([P, 1024], F32, tag="ps")
                nc.tensor.matmul(pp[:, 0:D], ones_row[0:1, 0:P],
                                 v_row[vec][0:1, :], start=True, stop=True)
                nc.vector.tensor_copy(t[:], pp[:, 0:D])
                bcast[vec] = t

            # ---- transpose x into [D, seq] layout (two 128-row tiles) ------
            xT = [io.tile([P, seq], F32R, tag=f"xT{i}", name=f"xT{i}") for i in range(2)]
            xqT = [io.tile([P, sq], F32R, tag=f"xqT{i}", name=f"xqT{i}") for i in range(2)]
            for t in range(NK):
                for dblk in range(2):
                    pp = ps_s.tile([P, 1024], F32, tag="ps")
                    nc.tensor.transpose(pp[:, 0:P],
                                        x_sb[:, t, ts(dblk, P)], ident[:])
                    nc.vector.tensor_copy(xT[dblk][:, ts(t, P)], pp[:, 0:P])
            for t in range(NQ):
                for dblk in range(2):
                    pp = ps_et.tile([P, 1024], F32, tag="et")
                    nc.tensor.transpose(pp[:, 0:P],
                                        xq_sb[:, t, ts(dblk, P)], ident[:])
                    nc.vector.tensor_copy(xqT[dblk][:, ts(t, P)], pp[:, 0:P])

            # residual + output-projection bias, per q block
            xpb = io.tile([P, NQ, D], F32, tag="xpb")
            for j in range(NQ):
                nc.vector.tensor_tensor(xpb[:, j, :], xq_sb[:, j, :],
                                        bcast["bo"][:], op=mybir.AluOpType.add)

            for rep in range(repeats):
                # ---- projections (paired heads: pair p covers heads 2p,2p+1)
                QT = [io.tile([P, sq], F32R, tag=f"QT{i}", name=f"QT{i}") for i in range(2)]
                KT = [io.tile([P, seq], F32R, tag=f"KT{i}", name=f"KT{i}") for i in range(2)]
                V = [io.tile([P, NK * P], F32R, tag=f"V{i}", name=f"V{i}") for i in range(2)]
                ctxT = [io.tile([P, sq], F32R, tag=f"ctxT{i}", name=f"ctxT{i}") for i in range(2)]

                def proj_qk(pair):
                    for c in range(QC):
                        pp = ps_s.tile([P, 1024], F32, tag="ps")
                        for dblk in range(2):
                            nc.tensor.matmul(
                                pp[:, 0:512],
                                w_sb["Wq"][:, dblk, ts(pair, P)],
                                xqT[dblk][:, ts(c, 512)],
                                start=(dblk == 0), stop=(dblk == 1))
                        nc.vector.tensor_scalar(
                            QT[pair][:, ts(c, 512)], pp[:, 0:512],
                            bqT[:, pair:pair + 1], None,
                            op0=mybir.AluOpType.add)
                    for c in range(KC):
                        pp = ps_s.tile([P, 1024], F32, tag="ps")
                        for dblk in range(2):
                            nc.tensor.matmul(
                                pp[:, 0:512],
                                w_sb["Wk"][:, dblk, ts(pair, P)],
                                xT[dblk][:, ts(c, 512)],
                                start=(dblk == 0), stop=(dblk == 1))
                        nc.vector.tensor_scalar(
                            KT[pair][:, ts(c, 512)], pp[:, 0:512],
                            bkT[:, pair:pair + 1], None,
                            op0=mybir.AluOpType.add)

                def proj_v(pair):
                    for t in range(NK):
                        pp = ps_s.tile([P, 1024], F32, tag="ps")
                        for dblk in range(2):
                            nc.tensor.matmul(pp[:, 0:P],
                                             xT[dblk][:, ts(t, P)],
                                             w_sb["Wv"][:, dblk, ts(pair, P)],
                                             start=(dblk == 0), stop=(dblk == 1))
                        nc.vector.tensor_tensor(V[pair][:, ts(t, P)], pp[:, 0:P],
                                                bcast["bv"][:, ts(pair, P)],
                                                op=mybir.AluOpType.add)

                recs = {}

                def e_phase(h):
                    pair, base = h // 2, (h % 2) * HD
                    for j in range(NQ):
                        at = attnp.tile([P, seq], F32, tag="at")
                        ds_t = stat.tile([P, KC], F32, tag="dsum")
                        for kc2 in range(KC // 2):
                            pp = ps_s.tile([P, 1024], F32, tag="ps")
                            for c in range(2):
                                kc = kc2 * 2 + c
                                nc.tensor.matmul(
                                    pp[:, ts(c, 512)],
                                    QT[pair][base:base + HD, ts(j, P)],
                                    KT[pair][base:base + HD, ts(kc, 512)],
                                    start=True, stop=True,
                                    tile_position=(base, 0))
                            nc.scalar.activation(
                                at[:, ts(kc2, 1024)], pp[:],
                                mybir.ActivationFunctionType.Exp,
                                scale=0.125,
                                accum_out=ds_t[:, kc2:kc2 + 1])
                        rc = recp.tile([P, 1], F32, tag=f"rc{h}_{j}")
                        dn = stat.tile([P, 1], F32, tag="dn")
                        nc.vector.reduce_sum(dn[:], ds_t[:],
                                             axis=mybir.AxisListType.X)
                        nc.vector.reciprocal(rc[:], dn[:])
                        recs[(h, j)] = rc
                        nc.vector.tensor_scalar(at[:], at[:], rc[:], None,
                                                op0=mybir.AluOpType.mult)
                        nc.sync.dma_start(attn_d[h, ts(j, P), :], at[:])

                def et_ctx_phase(pair):
                    cps = [ps_s.tile([HD, sq], F32, tag="ps", name=f"cps{h2}")
                           for h2 in range(2)]

                    def emit_ctx(kblk, ets):
                        for h2 in range(2):
                            for c in range(QC):
                                nc.tensor.matmul(
                                    cps[h2][:, ts(c, 512)],
                                    V[pair][:, kblk * P + h2 * HD:
                                              kblk * P + h2 * HD + HD],
                                    ets[h2][:, ts(c, 512)],
                                    start=(kblk == 0), stop=(kblk == NK - 1))

                    # software pipeline: ctx(k) is emitted after scoresT(k+1)
                    # so the PE never stalls waiting on exp(k)
                    pending = None
                    for kblk in range(NK):
                        ets = []
                        for h2 in range(2):
                            base = h2 * HD
                            ep = ps_et.tile([P, 1024], F32, tag="et")
                            et_sb = etsp.tile([P, sq], F32R, tag="ets")
                            for c in range(QC):
                                nc.tensor.matmul(
                                    ep[:, ts(c, 512)],
                                    KT[pair][base:base + HD, ts(kblk, P)],
                                    QT[pair][base:base + HD, ts(c, 512)],
                                    start=True, stop=True,
                                    tile_position=(base, 0))
                            nc.scalar.activation(
                                et_sb[:], ep[:, 0:sq],
                                mybir.ActivationFunctionType.Exp, scale=0.125)
                            ets.append(et_sb)
                        if pending is not None:
                            emit_ctx(*pending)
                        pending = (kblk, ets)
                    emit_ctx(*pending)
                    for h2 in range(2):
                        nc.vector.tensor_copy(ctxT[pair][h2 * HD:(h2 + 1) * HD, :],
                                              cps[h2][:])

                # interleave: qk0 -> E(h0) ... with projections of the other
                # pair and V between, to keep ACT fed early
                proj_qk(0)
                e_phase(0)
                proj_qk(1)
                e_phase(1)
                proj_v(0)
                et_ctx_phase(0)
                e_phase(2)
                proj_v(1)
                e_phase(3)
                et_ctx_phase(1)

                # ---- output projection + residual + LayerNorm --------------
                yout = io.tile([P, NQ, D], F32, tag=f"yout")
                for j in range(NQ):
                    ya = yacp.tile([P, D], F32, tag="ya")
                    for h in range(H):
                        pair, base = h // 2, (h % 2) * HD
                        yp = ps_s.tile([P, 1024], F32, tag="ps")
                        nc.tensor.matmul(yp[:, 0:D],
                                         ctxT[pair][base:base + HD, ts(j, P)],
                                         w_sb["Wo"][base:base + HD, h // 2, :],
                                         start=True, stop=True,
                                         tile_position=(base, 0))
                        nc.vector.scalar_tensor_tensor(
                            ya[:], yp[:, 0:D], recs[(h, j)][:],
                            xpb[:, j, :] if h == 0 else ya[:],
                            op0=mybir.AluOpType.mult, op1=mybir.AluOpType.add)
                    st6 = stat.tile([P, 6], F32, tag="st6")
                    mv = stat.tile([P, 2], F32, tag="mv")
                    nc.vector.bn_stats(st6[:], ya[:])
                    nc.vector.bn_aggr(mv[:], st6[:])
                    rstd = stat.tile([P, 1], F32, tag="rstd")
                    nc.scalar.activation(rstd[:], mv[:, 1:2],
                                         mybir.ActivationFunctionType.Sqrt,
                                         bias=eps_t[:], scale=1.0)
                    nc.vector.reciprocal(rstd[:], rstd[:])
                    t1 = yacp.tile([P, D], F32, tag="t1")
                    nc.vector.tensor_scalar(t1[:], ya[:], mv[:, 0:1], rstd[:],
                                            op0=mybir.AluOpType.subtract,
                                            op1=mybir.AluOpType.mult)
                    nc.vector.tensor_tensor(t1[:], t1[:], bcast["gamma"][:],
                                            op=mybir.AluOpType.mult)
                    nc.vector.tensor_tensor(yout[:, j, :], t1[:],
                                            bcast["beta"][:],
                                            op=mybir.AluOpType.add)
                nc.sync.dma_start(out_d.rearrange("(t p) d -> p t d", p=P),
                                  yout[:])

    nc.compile()
    return nc


_PROG_CACHE = {}


def _get_program(seq=S, n_cores=N_CORES, repeats=1):
    key = (seq, n_cores, repeats)
    if key not in _PROG_CACHE:
        _PROG_CACHE[key] = build_program(seq, n_cores, repeats)
    return _PROG_CACHE[key]


def make_in_maps(x, Wq, bq, Wk, bk, Wv, bv, Wo, bo, gamma, beta, n_cores=N_CORES):
    x = np.ascontiguousarray(np.asarray(x, dtype=np.float32))
    seq = x.shape[1]
    sq = seq // 2
    shared = {
        "Wq": np.asarray(Wq, np.float32), "Wk": np.asarray(Wk, np.float32),
        "Wv": np.asarray(Wv, np.float32), "Wo": np.asarray(Wo, np.float32),
        "bq": np.asarray(bq, np.float32).reshape(1, D),
        "bk": np.asarray(bk, np.float32).reshape(1, D),
        "bv": np.asarray(bv, np.float32).reshape(1, D),
        "bo": np.asarray(bo, np.float32).reshape(1, D),
        "gamma": np.asarray(gamma, np.float32).reshape(1, D),
        "beta": np.asarray(beta, np.float32).reshape(1, D),
    }
    in_maps = []
    for c in range(n_cores):
        b, qh = c // 2, c % 2
        m = dict(shared)
        m["x"] = np.ascontiguousarray(x[b])
        m["xq"] = np.ascontiguousarray(x[b, qh * sq:(qh + 1) * sq])
        in_maps.append(m)
    return in_maps


def kernel(x, Wq, bq, Wk, bk, Wv, bv, Wo, bo, gamma, beta):
    x = np.asarray(x, np.float32)
    b_, seq, d_ = x.shape
    sq = seq // 2
    nc = _get_program(seq, N_CORES, 1)
    in_maps = make_in_maps(x, Wq, bq, Wk, bk, Wv, bv, Wo, bo, gamma, beta)
    res = run_bass_kernel_spmd(nc, in_maps, list(range(N_CORES)))
    out = np.empty((b_, seq, d_), np.float32)
    attn = np.empty((b_, H, seq, seq), np.float32)
    for c in range(N_CORES):
        b, qh = c // 2, c % 2
        out[b, qh * sq:(qh + 1) * sq] = res.results[c]["out_p"]
        attn[b, :, qh * sq:(qh + 1) * sq, :] = res.results[c]["attn_p"]
    return out, attn
